# revision 6
# baseline (speedup 1.0000x reference)
"""Trainium2 Bass kernel v5 for nn_AttentionSubModule: PE-centric batched
tiny attention.

Per core: 16384 items = 128 tiles of 128 items; tile = 32 groups x 4 items
(b = 128t + 4g + m). Host pre-permutes x (pure layout staging):
  X~[t, 9m+d', G*j+g...] -> xt[t, 36, (g,j)]: xt[t, 9m+d', 25g+j] = x[b, 9j+d']
  XR[t, 25m+i, 9g+e]   = x[b, 9i+e]
and post-permutes the bf16 output back to [B, 25, 9] f32.

Device pipeline per tile:
 - Projections on PE per token-group h: stationary block-diag-over-m W_h
   [36,36] bf16, moving = xt cols j in h. Evac + per-partition bias ->
   Mq/Mk [36, (j,g)] bf16, MvT [36=(m,e), (j,g)] bf16.
 - v re-layout via DRAM round trip, supertile-batched: MvT -> dmv ->
   Mv [(m,j), (r,e,g)] bf16, ones plane at e=9 (memset) gives Z for free.
 - scores: 128 per-item PE matmuls  lhsT=Mk[9m:9m+9, (j,g=g)] [9,25],
   rhs=Mq[...] [9,25] -> sT psum [(m,j), (g,i)].
 - exp on ACT, bias AP = -8 - 1e9*mask[j] (mask + range shift folded).
 - attn@v: 128 per-item PE matmuls lhsT=ex[(m,j), (g-block i)] [25,25],
   rhs=Mv slice [25,10] -> u psum [(m,i), (g, e-aug)], Z at e=9.
 - residual + LayerNorm in [(m,i), (g,e)] layout on DVE/ACT/Pool
   (LN scale-invariance absorbs the softmax normalizer, as usual).
"""

import numpy as np
from contextlib import ExitStack

import concourse.bass as bass
import concourse.tile as tile
from concourse import mybir
from concourse.bass_utils import run_bass_kernel_spmd

KV = 9
NQ = 25
D = NQ * KV
GROUPS = [(0, 27, 3), (27, 117, 10), (117, 207, 10), (207, 225, 2)]
TOK_H = [(0, 3), (3, 13), (13, 23), (23, 25)]
TOKH_OF = {0: 0, 3: 1, 13: 2, 16: 2, 23: 3}
N_CORES = 8
M4 = 4
G = 32
EPS = 1e-5
F32 = mybir.dt.float32
BF16 = mybir.dt.float16
R_SUP = 8

PM_COLS = 4 * (128 + 128 + 36)
FB_COLS = 12


def build_program_v5(T, gb_generic=False, probe=4):
    assert T % R_SUP == 0
    ST = T // R_SUP
    nc = bass.Bass("TRN2", target_bir_lowering=False)

    xt_d = nc.dram_tensor("xt", [T, 37, G * NQ], BF16, kind="ExternalInput")
    xr_d = nc.dram_tensor("xr", [T, 128, G * KV], BF16, kind="ExternalInput")
    pm_d = nc.dram_tensor("pm", [37, PM_COLS], BF16, kind="ExternalInput")
    eb_d = nc.dram_tensor("eb", [128, 1], F32, kind="ExternalInput")
    gb_d = nc.dram_tensor("gb", [128, 2 * KV], BF16, kind="ExternalInput")
    o_d = nc.dram_tensor("o", [T, 128, G * KV], BF16, kind="ExternalOutput")

    AF = mybir.ActivationFunctionType

    with tile.TileContext(nc) as tc, ExitStack() as ctx:
        consts = ctx.enter_context(tc.tile_pool(name="consts", bufs=1))
        sup = ctx.enter_context(tc.tile_pool(name="sup", bufs=2))
        dram = ctx.enter_context(tc.tile_pool(name="dram", bufs=2, space="DRAM"))
        proj = ctx.enter_context(tc.tile_pool(name="proj", bufs=3))
        expp = ctx.enter_context(tc.tile_pool(name="exsb", bufs=2 * R_SUP + 2))
        lnp = ctx.enter_context(tc.tile_pool(name="lnp", bufs=3))
        pproj = ctx.enter_context(tc.tile_pool(name="pproj", bufs=2, space="PSUM"))
        psc = ctx.enter_context(tc.tile_pool(name="psc", bufs=2, space="PSUM"))
        pu = ctx.enter_context(tc.tile_pool(name="pu", bufs=2, space="PSUM"))

        pm_t = consts.tile([37, PM_COLS], BF16)
        nc.sync.dma_start(out=pm_t, in_=pm_d[:, :])
        eb_t = consts.tile([128, 1], F32)
        nc.sync.dma_start(out=eb_t, in_=eb_d[:, :])
        gb_t = consts.tile([128, 2 * KV], BF16)
        nc.sync.dma_start(out=gb_t, in_=gb_d[:, :])

        Wmat = {}
        for h in range(4):
            c = (128 + 128 + 36) * h
            Wmat["q", h] = pm_t[:, c : c + 128]
            Wmat["k", h] = pm_t[:, c + 128 : c + 256]
            Wmat["v", h] = pm_t[:, c + 256 : c + 292]

        inv3 = float(1.0 / np.sqrt(KV))

        for s in range(ST):
            xts = sup.tile([37, R_SUP * G * NQ], BF16, tag="xts")
            nc.sync.dma_start(
                out=xts[:].rearrange("p (r c) -> p r c", r=R_SUP),
                in_=xt_d[s * R_SUP : (s + 1) * R_SUP, :, :].transpose([1, 0, 2]),
            )
            xrs = sup.tile([128, R_SUP * G * KV], BF16, tag="xrs")
            nc.sync.dma_start(
                out=xrs[:].rearrange("p (r c) -> p r c", r=R_SUP),
                in_=xr_d[s * R_SUP : (s + 1) * R_SUP, :, :].transpose([1, 0, 2]),
            )
            mvts = sup.tile([36, R_SUP * NQ * G], BF16, tag="mvts")
            os_t = sup.tile([128, R_SUP * G * KV], BF16, tag="os")

            exs = []
            # ---------- pass 1: projections, scores, exp ----------
            for r in range(R_SUP):
                xv = xts[:, r * G * NQ : (r + 1) * G * NQ].rearrange(
                    "p (g j) -> p g j", g=G)

                mq_t = proj.tile([128, NQ * G], BF16, tag="mq")
                mk_t = proj.tile([128, NQ * G], BF16, tag="mk")
                mvt = mvts[:, r * NQ * G : (r + 1) * NQ * G]
                # q/k/v sequentially through one 2-bank psum tag.
                # q/k psum layout: col = 25*g + j for g<16, 512 + 25*(g-16) + j
                # v psum layout: col = 32*j + g (contiguous across banks at j=16)
                for nm in ("q", "k", "v"):
                    rows = 36 if nm == "v" else 128
                    pp = pproj.tile([128, 1024], F32, tag="pp")
                    if nm == "v":
                        # split by j at the bank boundary (j=16)
                        for j0, j1 in ((0, 3), (3, 13), (13, 16),
                                       (16, 23), (23, 25)):
                            w = j1 - j0
                            rhs = xts[:, r * G * NQ : (r + 1) * G * NQ].rearrange(
                                "p (g j) -> p g j", g=G)[:, :, j0:j1]
                            dst = bass.AP(
                                tensor=pp[:].tensor,
                                offset=pp[:].offset + 32 * j0,
                                ap=[[pp[:].ap[0][0], rows], [1, G], [G, w]],
                            )
                            nc.tensor.matmul(dst, Wmat[nm, TOKH_OF[j0]], rhs,
                                             start=True, stop=True)
                        nc.scalar.copy(mvt, pp[0:36, 0:800])
                    else:
                        for gh in range(2):
                            for h in range(4):
                                j0, j1 = TOK_H[h]
                                w = j1 - j0
                                rhs = xts[:, r * G * NQ : (r + 1) * G * NQ
                                          ].rearrange("p (g j) -> p g j", g=G)[
                                    :, 16 * gh : 16 * gh + 16, j0:j1]
                                dst = bass.AP(
                                    tensor=pp[:].tensor,
                                    offset=pp[:].offset + 512 * gh + j0,
                                    ap=[[pp[:].ap[0][0], rows], [NQ, 16], [1, w]],
                                )
                                nc.tensor.matmul(dst, Wmat[nm, h], rhs,
                                                 start=True, stop=True)
                        mdst = (mq_t if nm == "q" else mk_t)[:].rearrange(
                            "p (gh c) -> p gh c", gh=2)
                        msrc = bass.AP(
                            tensor=pp[:].tensor, offset=pp[:].offset,
                            ap=[[pp[:].ap[0][0], 128], [512, 2], [1, 400]],
                        )
                        if nm == "q":
                            nc.scalar.copy(mdst, msrc)
                        else:
                            nc.vector.tensor_copy(mdst, msrc)

                if probe <= 1:
                    nc.vector.tensor_copy(
                        os_t[:, r * G * KV : (r + 1) * G * KV],
                        mq_t[:, 0 : G * KV])
                    continue
                # scores: one 2-bank psum, g<16 at cols 25g+j, g>=16 at
                # 512 + 25(g-16) + j
                sc = psc.tile([128, 1024], F32, tag="sc", bufs=1)
                mk3 = mk_t[:].rearrange("p (g j) -> p g j", g=G)
                mq3 = mq_t[:].rearrange("p (g j) -> p g j", g=G)
                for g in range(G):
                    c0 = 25 * g if g < 16 else 512 + 25 * (g - 16)
                    for m in range(M4):
                        nc.tensor.matmul(
                            sc[32 * m : 32 * m + 25, c0 : c0 + 25],
                            mk3[32 * m : 32 * m + 9, g, :],
                            mq3[32 * m : 32 * m + 9, g, :],
                            start=True, stop=True,
                            tile_position=(32 * m, 32 * m))

                ex_t = expp.tile([128, G * NQ], BF16, tag="ex")
                nc.scalar.activation(ex_t[:, 0:400], sc[:, 0:400], AF.Exp,
                                     bias=eb_t[:], scale=inv3)
                nc.scalar.activation(ex_t[:, 400:800], sc[:, 512:912], AF.Exp,
                                     bias=eb_t[:], scale=inv3)
                exs.append(ex_t)
                if probe <= 2:
                    nc.vector.tensor_copy(
                        os_t[:, r * G * KV : (r + 1) * G * KV],
                        ex_t[:, 0 : G * KV])

            if probe <= 2:
                nc.sync.dma_start(
                    out=o_d[s * R_SUP : (s + 1) * R_SUP, :, :].transpose([1, 0, 2]),
                    in_=os_t[:].rearrange("p (r c) -> p r c", r=R_SUP),
                )
                continue
            RH = R_SUP // 4
            for hf in range(4):
                # ---------- v round trip (half supertile) ----------
                # dmv element layout: off = 28800 rh + 1152 j + 288 m + 32 e + g
                dmv = dram.tile([36, RH * NQ * G], BF16, tag=f"dmv{hf}")
                dmv_ap = dmv[:]
                dump_dst = bass.AP(
                    tensor=dmv_ap.tensor, offset=dmv_ap.offset,
                    ap=[[32, 36], [1152, RH * NQ], [1, G]],
                )
                mvh = mvts[:, hf * RH * NQ * G : (hf + 1) * RH * NQ * G]
                nc.sync.dma_start(out=dump_dst, in_=mvh.rearrange(
                    "p (rj g) -> p rj g", g=G))
                mv_s = sup.tile([128, RH * 10 * G], BF16, tag=f"mvs{hf}")
                mv4 = mv_s[:].rearrange("p (r e g) -> p r e g", r=RH, e=10)
                nc.vector.memset(mv4[:, :, 9, :], 1.0)
                for m in range(M4):
                    dstv = mv4[32 * m : 32 * m + 25, :, 0:9, :]
                    srcv = bass.AP(
                        tensor=dmv_ap.tensor,
                        offset=dmv_ap.offset + 288 * m,
                        ap=[[1152, NQ], [28800, RH], [1, KV * G]],
                    )
                    nc.sync.dma_start(out=dstv, in_=srcv)

                # ---------- pass 2 on this half (tile pairs) ----------
                for r2 in range(RH // 2):
                    u_s = lnp.tile([128, 2, 320], F32, tag="us")
                    for half in range(2):
                        rh = 2 * r2 + half
                        r = hf * RH + rh
                        ex_t = exs[r]
                        u_ps = pu.tile([128, 320], F32, tag="u")
                        mv_r = mv4[:, rh, :, :]
                        for g in range(G):
                            e0 = 25 * g if g < 16 else 400 + 25 * (g - 16)
                            for m in range(M4):
                                nc.tensor.matmul(
                                    u_ps[32 * m : 32 * m + 25,
                                         10 * g : 10 * g + 10],
                                    ex_t[32 * m : 32 * m + 25, e0 : e0 + 25],
                                    mv_r[32 * m : 32 * m + 25, :, g],
                                    start=True, stop=True,
                                    tile_position=(32 * m, 32 * m))
                        nc.scalar.copy(u_s[:, half, :], u_ps[:])

                    r = hf * RH + 2 * r2
                    u3 = u_s[:].rearrange("p h (g e) -> p (h g) e", g=G)
                    uvec = u3[:, :, 0:9]
                    z_ap = u3[:, :, 9:10]
                    G2 = 2 * G
                    xr_r = xrs[:, r * G * KV : (r + 2) * G * KV].rearrange(
                        "p (g e) -> p g e", g=G2)

                    zx = lnp.tile([128, G2, KV], F32, tag="zx")
                    nc.gpsimd.tensor_mul(zx[:], xr_r,
                                         z_ap.broadcast_to((128, G2, KV)))
                    u2 = lnp.tile([128, G2, KV], F32, tag="u2")
                    nc.gpsimd.tensor_add(u2[:], uvec, zx[:])
                    s_t = lnp.tile([128, G2], F32, tag="s")
                    nc.vector.tensor_reduce(
                        s_t[:], u2[:], axis=mybir.AxisListType.X,
                        op=mybir.AluOpType.add)
                    mu = lnp.tile([128, G2], F32, tag="mu")
                    nc.gpsimd.tensor_scalar_mul(mu[:], s_t[:], 1.0 / KV)
                    cen = lnp.tile([128, G2, KV], F32, tag="cen")
                    nc.vector.tensor_sub(
                        cen[:], u2[:],
                        mu[:].unsqueeze(2).broadcast_to((128, G2, KV)))
                    sq = lnp.tile([128, G2, KV], F32, tag="sq")
                    nc.gpsimd.tensor_mul(sq[:], cen[:], cen[:])
                    vs = lnp.tile([128, G2], F32, tag="vs")
                    nc.vector.tensor_reduce(
                        vs[:], sq[:], axis=mybir.AxisListType.X,
                        op=mybir.AluOpType.add)
                    zsq = lnp.tile([128, G2], F32, tag="zsq")
                    zf = z_ap.rearrange("p a b -> p (a b)")
                    nc.gpsimd.tensor_mul(zsq[:], zf, zf)
                    vs2 = lnp.tile([128, G2], F32, tag="vs2")
                    nc.vector.scalar_tensor_tensor(
                        vs2[:], zsq[:], float(KV * EPS), vs[:],
                        op0=mybir.AluOpType.mult, op1=mybir.AluOpType.add)
                    sd = lnp.tile([128, G2], F32, tag="sd")
                    nc.scalar.activation(sd[:], vs2[:], AF.Sqrt,
                                         bias=0.0, scale=1.0 / KV)
                    rstd = lnp.tile([128, G2], F32, tag="rstd")
                    nc.vector.reciprocal(rstd[:], sd[:])

                    o_r = os_t[:, r * G * KV : (r + 2) * G * KV].rearrange(
                        "p (g e) -> p g e", g=G2)
                    if gb_generic:
                        o1 = lnp.tile([128, G2, KV], F32, tag="o1")
                        nc.vector.tensor_mul(
                            o1[:], cen[:],
                            rstd[:].unsqueeze(2).broadcast_to((128, G2, KV)))
                        gam = gb_t[:, 0:KV].unsqueeze(1).broadcast_to(
                            (128, G2, KV))
                        bet = gb_t[:, KV : 2 * KV].unsqueeze(1).broadcast_to(
                            (128, G2, KV))
                        o2 = lnp.tile([128, G2, KV], F32, tag="o2")
                        nc.vector.tensor_mul(o2[:], o1[:], gam)
                        nc.vector.tensor_add(o_r, o2[:], bet)
                    else:
                        nc.vector.tensor_mul(
                            o_r, cen[:],
                            rstd[:].unsqueeze(2).broadcast_to((128, G2, KV)))

            nc.sync.dma_start(
                out=o_d[s * R_SUP : (s + 1) * R_SUP, :, :].transpose([1, 0, 2]),
                in_=os_t[:].rearrange("p (r c) -> p r c", r=R_SUP),
            )

    _split_multi_waits(nc)
    return nc


def _split_multi_waits(nc):
    for f in nc.m.functions:
        for b in f.blocks:
            i = 0
            while i < len(b.instructions):
                inst = b.instructions[i]
                si = getattr(inst, "sync_info", None)
                if si is not None and si.on_wait and len(si.on_wait) > 1:
                    extra = si.on_wait[:-1]
                    si.on_wait = si.on_wait[-1:]
                    for w in extra:
                        nop = mybir.InstNoOp(
                            name=nc.get_next_instruction_name(),
                            engine=inst.engine, ins=[], outs=[],
                            sync_info=mybir.SyncInfo(on_wait=[w], on_update=[]),
                        )
                        nc.register_instruction(nop)
                        b.instructions.insert(i, nop)
                        i += 1
                i += 1
    return nc


# ---------------- host side ----------------

def _group_mats(W, b_):
    """Expand per-group (4) mats to per-token-group-h lists."""
    W = np.asarray(W, np.float32)
    b_ = np.asarray(b_, np.float32)
    return [W[h] for h in range(4)], [b_[h] for h in range(4)]


def _host_consts(Wq, bq, Wk, bk, Wv, bv, mask, gamma, beta):
    pm = np.zeros((37, PM_COLS), np.float32)
    for h in range(4):
        c = (128 + 128 + 36) * h
        Wqh = np.asarray(Wq, np.float32)[h]
        Wkh = np.asarray(Wk, np.float32)[h]
        Wvh = np.asarray(Wv, np.float32)[h]
        for m in range(M4):
            pm[9 * m : 9 * m + 9, c + 32 * m : c + 32 * m + 9] = Wqh.T
            pm[9 * m : 9 * m + 9, c + 128 + 32 * m : c + 128 + 32 * m + 9] = Wkh.T
            pm[9 * m : 9 * m + 9, c + 256 + 9 * m : c + 256 + 9 * m + 9] = Wvh.T
            pm[36, c + 32 * m : c + 32 * m + 9] = np.asarray(bq, np.float32)[h]
            pm[36, c + 128 + 32 * m : c + 128 + 32 * m + 9] = np.asarray(
                bk, np.float32)[h]
            pm[36, c + 256 + 9 * m : c + 256 + 9 * m + 9] = np.asarray(
                bv, np.float32)[h]

    eb = np.full((128, 1), -8.0, np.float32)
    mk = np.asarray(mask, np.float32)
    for m in range(M4):
        for j in range(NQ):
            eb[32 * m + j, 0] = -8.0 - 1e9 * float(mk[j])
    gb = np.concatenate([
        np.broadcast_to(np.asarray(gamma, np.float32), (128, KV)),
        np.broadcast_to(np.asarray(beta, np.float32), (128, KV)),
    ], axis=1)
    return pm, eb, np.ascontiguousarray(gb)


def host_layouts(x, T):
    xt4 = x.reshape(T, G, M4, NQ, KV)  # [t, g, m, j, d]
    xtil = np.ones((T, 37, G * NQ), np.float32)
    xtil[:, 0:36] = xt4.transpose(0, 2, 4, 1, 3).reshape(T, 36, G * NQ)
    xr = np.zeros((T, 128, G * KV), np.float16)
    xr4 = xt4.transpose(0, 2, 3, 1, 4).reshape(T, M4, NQ, G * KV)
    for m in range(M4):
        xr[:, 32 * m : 32 * m + 25, :] = xr4[:, m].astype(np.float16)
    return xtil.astype(np.float16), xr


def unpermute_out(o, T):
    o6 = o.reshape(T, 4, 32, G, KV)[:, :, :NQ]  # [t, m, i, g, e]
    o5 = o6.transpose(0, 3, 1, 2, 4)  # [t, g, m, i, e]
    return np.ascontiguousarray(o5).reshape(T * 128, NQ, KV).astype(np.float32)


_NC_CACHE = {}


def _get_program(T, gb_generic):
    key = (T, gb_generic)
    if key not in _NC_CACHE:
        _NC_CACHE[key] = build_program_v5(T, gb_generic)
    return _NC_CACHE[key]


def kernel(x, mask, Wq, bq, Wk, bk, Wv, bv, gamma, beta):
    x = np.ascontiguousarray(np.asarray(x, dtype=np.float32))
    B = x.shape[0]
    b_core = B // N_CORES
    T = b_core // 128
    gb_generic = not (np.all(np.asarray(gamma) == 1.0)
                      and np.all(np.asarray(beta) == 0.0))
    pm, eb, gb = _host_consts(Wq, bq, Wk, bk, Wv, bv, mask, gamma, beta)
    nc = _get_program(T, gb_generic)

    shards = x.reshape(N_CORES, b_core, D)
    in_maps = []
    for c in range(N_CORES):
        xtil, xr = host_layouts(shards[c], T)
        in_maps.append({
            "xt": xtil, "xr": xr,
            "pm": pm.astype(np.float16), "eb": eb,
            "gb": gb.astype(np.float16),
        })
    res = run_bass_kernel_spmd(nc, in_maps, core_ids=list(range(N_CORES)))
    outs = [unpermute_out(np.asarray(res.results[c]["o"], np.float32), T)
            for c in range(N_CORES)]
    return np.concatenate(outs, axis=0)
